# revision 17
# baseline (speedup 1.0000x reference)
"""Trainium2 Bass kernel for nn_CorrBlock: softmax(fmap1 @ fmap2.T / sqrt(D), axis=-1).

Sharding: fmap1 rows split across 8 cores (1024 rows each), fmap2 replicated.
Each core computes its [1024, 8192] slab of the output independently.

Device kernel (per core):
  - Inputs are pre-transposed on the host to [128, D/128, rows] so the
    contraction dim lands on SBUF partitions with no on-device transpose.
  - PE: matmuls accumulate the D=256 contraction in 2 chunks of 128 into PSUM.
  - ACT: Exp with fused 1/sqrt(D) scale reads PSUM, writes fp16 SBUF, and
    emits per-row partial sums via accum_out (f32). Row sums as DVE reduces
    were tried and regressed hard: TENSOR_REDUCE gets no 2x/4x DVE mode
    (~2.3us per 2048-chunk, 74us total) and overloaded DVE.
  - DVE: reciprocal of the row sum, then per-row scalar multiplies in fp16
    (4x mode), one per 2048-wide chunk, each followed by its 512KB store so
    the output streams out as it is produced. Whole-block 2MB stores were
    tried and pushed the final drain ~5us later.
  - DMA out fp16; host converts to f32. (fp16 halves output HBM traffic vs
    the f32 baseline, which was DMA-bound at 82%; softmax values round-trip
    fp16 at ~5e-4 rel err, well inside the 2e-2 gate.)

Pipeline shaping (from ntff traces of earlier revisions):
  - Input loads stay coarse (5 serial DMAs on one queue, 4KB descriptors).
    Splitting f2 into 1024-col pieces cost 4us (2KB descriptors dropped input
    DMA from ~276 to ~199 GB/s), and issuing chunks on a second HWDGE queue
    delayed the critical first chunk by sharing bandwidth (+4us). The serial
    order is balanced: the stream end is pinned equally by q0's arrival
    (+32 chunk slots) and q3's arrival (+29 slots).
"""

import os
import sys

import numpy as np

if "/opt/trn_rl_repo" not in sys.path:
    sys.path.insert(0, "/opt/trn_rl_repo")

import concourse.bacc as bacc
import concourse.bass as bass
import concourse.mybir as mybir
import concourse.tile as tile
from concourse.bass_utils import run_bass_kernel_spmd

N, M, D = 8192, 8192, 256
N_CORES = 8
NB = N // N_CORES  # rows per core
DC = D // 128  # contraction chunks
QC = 2048  # columns handled per PSUM tile (4 banks)

# Matmul input dtype: "float16" halves input DMA bytes and doubles PE rate
# vs "float32r", at ~5e-4 softmax rel err (vs ~2e-4). Both are far inside
# tolerance; float16 wins on the DMA roofline.
MM_DT = os.environ.get("CORR_MM_DT", "float16")

# Populated by kernel() on every run (exec_time_ns only when tracing).
last_run_info: dict = {}


def build_nc(nb=NB, m=M, dc=DC, qc=QC, mm_dt=None, exp_bufs=4):
    """Build the per-core Bass program. Shapes in elements."""
    f32 = mybir.dt.float32
    f16 = mybir.dt.float16
    mm_dtype = getattr(mybir.dt, mm_dt or MM_DT)
    n_blocks = nb // 128
    n_q = m // qc  # PSUM-sized column chunks per row block
    n_j = qc // 512  # 512-wide matmul tiles per chunk (ISA caps matmul moving dim at 512)
    scale = 1.0 / (D**0.5)

    nc = bacc.Bacc("TRN2", target_bir_lowering=False, debug=False)

    f1t = nc.dram_tensor("f1t", [128, dc, nb], mm_dtype, kind="ExternalInput")
    f2t = nc.dram_tensor("f2t", [128, dc, m], mm_dtype, kind="ExternalInput")
    out = nc.dram_tensor("out", [nb, m], f16, kind="ExternalOutput")

    with tile.TileContext(nc) as tc:
        with (
            tc.tile_pool(name="weights", bufs=1) as wpool,
            tc.tile_pool(name="exps", bufs=exp_bufs) as epool,
            tc.tile_pool(name="stats", bufs=2) as spool,
            tc.tile_pool(name="psum", bufs=2, space="PSUM") as ppool,
        ):
            f1s = wpool.tile([128, dc, nb], mm_dtype, tag="f1s")
            nc.sync.dma_start(f1s[:], f1t[:])
            f2s = []
            for q in range(n_q):
                f2q = wpool.tile(
                    [128, dc, qc], mm_dtype, tag=f"f2q_{q}", name=f"f2q_{q}"
                )
                nc.sync.dma_start(f2q[:], f2t[:, :, q * qc : (q + 1) * qc])
                f2s.append(f2q)

            for b in range(n_blocks):
                exps = epool.tile([128, m], f16, tag="exps", name=f"exps_{b}")
                sums = spool.tile([128, n_q], f32, tag="sums", name=f"sums_{b}")
                rsum = spool.tile([128, 1], f32, tag="rsum", name=f"rsum_{b}")
                recip = spool.tile([128, 1], f32, tag="recip", name=f"recip_{b}")
                for q in range(n_q):
                    ps = ppool.tile([128, n_j, 512], f32, tag="ps", name=f"ps_{b}_{q}")
                    for d in range(dc):
                        lhsT = f1s[:, d, b * 128 : (b + 1) * 128]
                        for j in range(n_j):
                            nc.tensor.matmul(
                                ps[:, j, :],
                                lhsT,
                                f2s[q][:, d, j * 512 : (j + 1) * 512],
                                start=(d == 0),
                                stop=(d == dc - 1),
                            )
                    nc.scalar.activation(
                        exps[:, q * qc : (q + 1) * qc],
                        ps.rearrange("p a b -> p (a b)"),
                        mybir.ActivationFunctionType.Exp,
                        scale=scale,
                        accum_out=sums[:, q : q + 1],
                    )
                nc.vector.reduce_sum(rsum[:], sums[:], axis=mybir.AxisListType.X)
                nc.vector.reciprocal(recip[:], rsum[:])
                for q in range(n_q):
                    sl = slice(q * qc, (q + 1) * qc)
                    nc.vector.tensor_scalar_mul(exps[:, sl], exps[:, sl], recip[:])
                    nc.sync.dma_start(out[b * 128 : (b + 1) * 128, sl], exps[:, sl])

    nc.compile()
    return nc


_nc_cache: dict = {}


def _get_nc():
    key = MM_DT
    if key not in _nc_cache:
        _nc_cache[key] = build_nc()
    return _nc_cache[key]


def kernel(fmap1: np.ndarray, fmap2: np.ndarray) -> np.ndarray:
    f1 = np.asarray(fmap1, dtype=np.float32)
    f2 = np.asarray(fmap2, dtype=np.float32)
    np_mm = mybir.dt.np(getattr(mybir.dt, MM_DT))
    # [rows, D] -> [128, D/128, rows]: f1t[dp, dcc, n] = f1[n, dcc*128 + dp]
    f1t = np.ascontiguousarray(
        f1.T.reshape(DC, 128, N).transpose(1, 0, 2).astype(np_mm)
    )
    f2t = np.ascontiguousarray(
        f2.T.reshape(DC, 128, M).transpose(1, 0, 2).astype(np_mm)
    )

    nc = _get_nc()
    in_maps = [
        {"f1t": np.ascontiguousarray(f1t[:, :, i * NB : (i + 1) * NB]), "f2t": f2t}
        for i in range(N_CORES)
    ]
    trace = bool(os.environ.get("BASS_TRACE"))
    res = run_bass_kernel_spmd(nc, in_maps, list(range(N_CORES)), trace=trace)
    last_run_info.clear()
    last_run_info.update(
        exec_time_ns=res.exec_time_ns,
        mean_exec_time_ns=res.mean_exec_time_ns,
        profile_json=res.profile_json,
        trace_path=(res.instructions_and_trace or (None, None))[1],
    )
    return np.concatenate(
        [res.results[i]["out"] for i in range(N_CORES)], axis=0
    ).astype(np.float32)
